# revision 1
# baseline (speedup 1.0000x reference)
"""DDNLoss (depth distribution network focal loss) on 8 trn2 NeuronCores.

Data-parallel over B (1 image per core, B=8). Each core:
  1. Rasterizes its 32 boxes into a min-depth map (96,312):
     per box, PE broadcasts the box's column-value row (depth where the
     box covers the column, +inf elsewhere) to 96 partitions via a K=1
     fp32 matmul (exact), then one fused DVE op does
     dmin = min(dmin, max(colval_bcast, rowpen_scalar)).
  2. Computes LID bin targets t(h,w) and fg weights on-chip, bounces
     them through DRAM to a (128, 234) pixel-partition "slot" layout
     (slot (i,g) <-> pixel 128*g + i).
  3. Streams depth_logits (81, 29952) in 13 contiguous chunks. Each
     128-pixel group is PE-transposed to (128, 81); ACT computes exp
     with accum_out giving sum_c exp directly; one fused DVE op
     (iota==t)*exp with accum_out gives exp(logit[target]).
  4. Focal-loss epilogue on (128,234) + free-dim accumulation ->
     per-partition partial sums (128,1).
Host sums the 8x128 partials (the "all-reduce") -> scalar loss.
"""

import numpy as np
from contextlib import ExitStack

import concourse.bass as bass
import concourse.bacc as bacc_mod
import concourse.tile as tile
import concourse.mybir as mybir
from concourse import masks
from concourse.bass_utils import run_bass_kernel_spmd

# Problem constants (hardcoded per contract)
B, C, H, W, N = 8, 81, 96, 312, 32
HW = H * W                      # 29952
CHUNK = 2304                    # pixels per streamed chunk
NCHUNK = HW // CHUNK            # 13
GP = CHUNK // 128               # 18 pixel-groups of 128 per chunk
NGRP = HW // 128                # 234

ALPHA = 0.25
D_MIN, D_MAX, NUM_BINS = 0.001, 60.0, 80
BIN_SIZE = 2.0 * (D_MAX - D_MIN) / (NUM_BINS * (1 + NUM_BINS))
K1 = 8.0 / BIN_SIZE             # sqrt arg scale
B1 = 1.0 - K1 * D_MIN           # sqrt arg bias
BIG = 1.0e30
C0 = -ALPHA / float(B * HW)     # fold -alpha and global pixel normalizer
# float->int32 conversion rounding on DVE: True = round-to-nearest (cast
# idx-0.5), False = truncate (cast idx directly). Flip if probe mismatches.
CAST_RNE = True

LAST_RESULTS = None


def build_program(ablate=()):
    f32 = mybir.dt.float32
    i32 = mybir.dt.int32
    Alu = mybir.AluOpType
    Act = mybir.ActivationFunctionType

    nc = bacc_mod.Bacc("TRN2", target_bir_lowering=False)
    logits = nc.dram_tensor("logits", [C, HW], f32, kind="ExternalInput")
    rowpen = nc.dram_tensor("rowpen", [H, N], f32, kind="ExternalInput")
    colval = nc.dram_tensor("colval", [N, W], f32, kind="ExternalInput")
    iotaf = nc.dram_tensor("iotaf", [128, C], f32, kind="ExternalInput")
    ones96 = nc.dram_tensor("ones96", [1, H], f32, kind="ExternalInput")
    partial = nc.dram_tensor("partial", [128, 1], f32, kind="ExternalOutput")
    tprobe = nc.dram_tensor("tprobe", [1, HW], f32, kind="ExternalOutput")

    with ExitStack() as ctx:
        tc = ctx.enter_context(tile.TileContext(nc))
        consts = ctx.enter_context(tc.tile_pool(name="consts", bufs=1))
        rast = ctx.enter_context(tc.tile_pool(name="rast", bufs=1))
        ts_pool = ctx.enter_context(tc.tile_pool(name="tstage", bufs=1))
        lg = ctx.enter_context(tc.tile_pool(name="lg", bufs=2))
        ex = ctx.enter_context(tc.tile_pool(name="ex", bufs=4))
        pr = ctx.enter_context(tc.tile_pool(name="pr", bufs=4))
        fin = ctx.enter_context(tc.tile_pool(name="fin", bufs=1))
        psb = ctx.enter_context(tc.tile_pool(name="psb", bufs=3, space="PSUM"))
        pst_pool = ctx.enter_context(tc.tile_pool(name="pst", bufs=4, space="PSUM"))
        dr = ctx.enter_context(tc.tile_pool(name="dr", bufs=1, space="DRAM"))

        # ---- constants
        zero128 = consts.tile([128, 1], f32)
        nc.vector.memset(zero128[:], 0.0)
        nc.const_aps.aps[(f32, 0.0)] = zero128[:]
        b1t = consts.tile([128, 1], f32)
        nc.vector.memset(b1t[:], B1)
        nc.const_aps.aps[(f32, B1)] = b1t[:]

        c_iota = consts.tile([128, C], f32)
        nc.sync.dma_start(c_iota[:], iotaf[:, :])
        c_ones96 = consts.tile([1, H], f32)
        nc.sync.dma_start(c_ones96[:], ones96[:, :])
        c_rowpen = consts.tile([H, N], f32)
        nc.sync.dma_start(c_rowpen[:], rowpen[:, :])
        c_cv = []
        for n in range(N):
            cvn = consts.tile([1, W], f32, tag=f"cv{n}")
            nc.sync.dma_start(cvn[:], colval[n:n + 1, :])
            c_cv.append(cvn)
        ident = consts.tile([128, 128], f32)
        masks.make_identity(nc, ident[:])

        # ---- rasterize: dmin(h,w) = min_n max(rowpen(h,n), colval(n,w))
        dmin = rast.tile([H, W], f32)
        nc.vector.memset(dmin[:], BIG)
        for n in range(N):
            bc = psb.tile([H, W], f32, tag="bc")
            nc.tensor.matmul(bc[:], c_ones96[:, :],
                             c_cv[n][0:1, :],
                             start=True, stop=True)
            # dmin = min(dmin, max(bc, rowpen[:, n]))
            nc.vector.scalar_tensor_tensor(
                out=dmin[:], in0=bc[:], scalar=c_rowpen[:, n:n + 1], in1=dmin[:],
                op0=Alu.max, op1=Alu.min)

        # ---- per-pixel targets in raster layout (96,312)
        fg = ts_pool.tile([H, W], f32)
        nc.vector.tensor_scalar(out=fg[:], in0=dmin[:], scalar1=BIG * 0.5,
                                scalar2=None, op0=Alu.is_lt)
        wgt = ts_pool.tile([H, W], f32)
        nc.vector.tensor_scalar(out=wgt[:], in0=fg[:], scalar1=12.0,
                                scalar2=1.0, op0=Alu.mult, op1=Alu.add)
        deff = ts_pool.tile([H, W], f32)
        nc.vector.tensor_tensor(out=deff[:], in0=dmin[:], in1=fg[:], op=Alu.mult)
        # idx = 0.5*sqrt(K1*d + B1) - 0.5
        sq = ts_pool.tile([H, W], f32)
        nc.scalar.activation(sq[:], deff[:], Act.Sqrt, bias=B1, scale=K1)
        idx = ts_pool.tile([H, W], f32)
        nc.vector.tensor_scalar(out=idx[:], in0=sq[:], scalar1=0.5,
                                scalar2=-0.5, op0=Alu.mult, op1=Alu.add)
        neg = ts_pool.tile([H, W], f32)
        nc.vector.tensor_scalar(out=neg[:], in0=idx[:], scalar1=0.0,
                                scalar2=None, op0=Alu.is_lt)
        idxc = ts_pool.tile([H, W], f32)
        if CAST_RNE:
            nc.vector.tensor_scalar(out=idxc[:], in0=idx[:], scalar1=80.0,
                                    scalar2=-0.5, op0=Alu.min, op1=Alu.add)
        else:
            nc.vector.tensor_scalar(out=idxc[:], in0=idx[:], scalar1=80.0,
                                    scalar2=None, op0=Alu.min)
        ti = ts_pool.tile([H, W], i32)
        nc.vector.tensor_copy(out=ti[:], in_=idxc[:])
        tf = ts_pool.tile([H, W], f32)
        nc.vector.tensor_copy(out=tf[:], in_=ti[:])
        # t = tf + neg*(80 - tf)   (idx<0 -> bin 80)
        d80 = ts_pool.tile([H, W], f32)
        nc.vector.tensor_scalar(out=d80[:], in0=tf[:], scalar1=-1.0,
                                scalar2=80.0, op0=Alu.mult, op1=Alu.add)
        nd = ts_pool.tile([H, W], f32)
        nc.vector.tensor_tensor(out=nd[:], in0=neg[:], in1=d80[:], op=Alu.mult)
        tt_ = ts_pool.tile([H, W], f32)
        nc.vector.tensor_tensor(out=tt_[:], in0=tf[:], in1=nd[:], op=Alu.add)

        nc.sync.dma_start(tprobe[0:1, :], tt_[:])

        # ---- bounce t and w to DRAM (flat pixel order), reload in slot
        # layout: slot (i, g) <- pixel 128*g + i
        tscr = dr.tile([NGRP, 128], f32)
        nc.sync.dma_start(tscr[:, :], tt_[:])
        wscr = dr.tile([NGRP, 128], f32)
        nc.sync.dma_start(wscr[:, :], wgt[:])
        t_slot = fin.tile([128, NGRP], f32)
        nc.sync.dma_start(t_slot[:], tscr[:, :].rearrange("c i -> i c"))
        w_slot = fin.tile([128, NGRP], f32)
        nc.sync.dma_start(w_slot[:], wscr[:, :].rearrange("c i -> i c"))

        # ---- stream logits; per 128-pixel group: PE transpose ->
        # (128, 81), exp+rowsum on ACT, (iota==t)*exp rowsum on DVE
        s128 = fin.tile([128, NGRP], f32)   # sum_c exp
        e128 = fin.tile([128, NGRP], f32)   # exp(logit[target])
        for j in range(NCHUNK):
            sl = slice(j * CHUNK, (j + 1) * CHUNK)
            L = lg.tile([C, CHUNK], f32, tag="L")
            nc.sync.dma_start(L[:], logits[:, sl])
            for k in range(GP):
                g = j * GP + k
                ksl = slice(k * 128, (k + 1) * 128)
                if "tp" in ablate:
                    continue
                pst = pst_pool.tile([128, C], f32, tag="pst")
                nc.tensor.transpose(pst[:], L[:, ksl], ident[0:C, 0:C])
                if "exp" in ablate:
                    continue
                expt = ex.tile([128, C], f32, tag="expt")
                nc.scalar.activation(expt[:], pst[:], Act.Exp,
                                     accum_out=s128[:, g:g + 1])
                if "prod" in ablate:
                    continue
                prod = pr.tile([128, C], f32, tag="prod")
                nc.vector.scalar_tensor_tensor(
                    out=prod[:], in0=c_iota[:], scalar=t_slot[:, g:g + 1],
                    in1=expt[:], op0=Alu.is_equal, op1=Alu.mult,
                    accum_out=e128[:, g:g + 1])

        # ---- focal epilogue on (128, 234)
        rS = fin.tile([128, NGRP], f32)
        nc.vector.reciprocal(rS[:], s128[:])
        p = fin.tile([128, NGRP], f32)
        nc.vector.tensor_tensor(out=p[:], in0=e128[:], in1=rS[:], op=Alu.mult)
        logp = fin.tile([128, NGRP], f32)
        nc.scalar.activation(logp[:], p[:], Act.Ln)
        om = fin.tile([128, NGRP], f32)
        nc.vector.tensor_scalar(out=om[:], in0=p[:], scalar1=-1.0,
                                scalar2=1.0, op0=Alu.mult, op1=Alu.add)
        om2 = fin.tile([128, NGRP], f32)
        nc.vector.tensor_tensor(out=om2[:], in0=om[:], in1=om[:], op=Alu.mult)
        t2 = fin.tile([128, NGRP], f32)
        nc.vector.scalar_tensor_tensor(
            out=t2[:], in0=om2[:], scalar=C0, in1=logp[:],
            op0=Alu.mult, op1=Alu.mult)
        fs = fin.tile([128, NGRP], f32)
        acc = fin.tile([128, 1], f32)
        nc.vector.scalar_tensor_tensor(
            out=fs[:], in0=t2[:], scalar=0.0, in1=w_slot[:],
            op0=Alu.add, op1=Alu.mult, accum_out=acc[:])
        nc.sync.dma_start(partial[:, :], acc[:])

    nc.compile()
    return nc


_CACHE = {}


def _get_program():
    if "nc" not in _CACHE:
        _CACHE["nc"] = build_program()
    return _CACHE["nc"]


def kernel(depth_logits, gt_boxes2d, num_gt_per_img, gt_center_depth):
    global LAST_RESULTS
    dl = np.ascontiguousarray(np.asarray(depth_logits, dtype=np.float32))
    assert dl.shape == (B, C, H, W), dl.shape
    n_gt = int(num_gt_per_img)
    assert n_gt == N, n_gt
    boxes = np.asarray(gt_boxes2d, dtype=np.float32)
    depth = np.asarray(gt_center_depth, dtype=np.float32)

    u1 = np.floor(boxes[:, 0]).astype(np.int32)
    v1 = np.floor(boxes[:, 1]).astype(np.int32)
    u2 = np.ceil(boxes[:, 2]).astype(np.int32)
    v2 = np.ceil(boxes[:, 3]).astype(np.int32)
    rows = np.arange(H)[:, None]
    cols = np.arange(W)[None, :]
    iota = np.ascontiguousarray(
        np.tile(np.arange(C, dtype=np.float32), (128, 1)))
    ones = np.ones((1, H), dtype=np.float32)

    logits_flat = dl.reshape(B, C, HW)
    in_maps = []
    for b in range(B):
        sl = slice(b * N, (b + 1) * N)
        bv1, bv2, bu1, bu2, d = v1[sl], v2[sl], u1[sl], u2[sl], depth[sl]
        rp = np.where((rows >= bv1[None, :]) & (rows < bv2[None, :]),
                      0.0, BIG).astype(np.float32)              # (H, N)
        cv = np.where((cols >= bu1[:, None]) & (cols < bu2[:, None]),
                      d[:, None], BIG).astype(np.float32)       # (N, W)
        in_maps.append({
            "logits": np.ascontiguousarray(logits_flat[b]),
            "rowpen": np.ascontiguousarray(rp),
            "colval": np.ascontiguousarray(cv),
            "iotaf": iota,
            "ones96": ones,
        })

    nc = _get_program()
    res = run_bass_kernel_spmd(nc, in_maps, core_ids=list(range(B)))
    LAST_RESULTS = res
    total = np.float64(0.0)
    for r in res.results:
        total += np.asarray(r["partial"], dtype=np.float64).sum()
    return np.float32(total)


if __name__ == "__main__":
    import tempfile
    from concourse.bass_utils import compile_bass_kernel
    compile_bass_kernel(_get_program(), tempfile.mkdtemp())
    print("COMPILE OK")



# revision 11
# speedup vs baseline: 1.3158x; 1.3158x over previous
"""DDNLoss (depth distribution network focal loss) on 8 trn2 NeuronCores.

v2 — natural-layout, batched. Data-parallel over B (1 image per core).

Per core:
  1. Rasterize 32 boxes into a min-depth map (96,312): per box PE
     broadcasts the box's column-value row (K=1 matmul), ACT adds the
     per-row penalty via per-partition bias (Relu, additive BIG
     sentinels, fp16), DVE runs a pure 32-op fp16 min chain.
  2. LID bin targets t(h,w) computed exactly as the torch reference
     (floor emulated by RNE cast of idx-0.5); t bounced to DRAM as a
     bf16 flat row.
  3. Logits streamed as bf16 (host-converted) in 8 chunks of (81,3744).
     Per chunk: ACT exp -> X (batched, 4x mode); one DVE
     scalar_tensor_tensor builds masked_L = (t_bcast==iota_c)*L where
     t_bcast is a 0-stride broadcast DMA of the t row; PE reduces both
     with a ones(81) stationary: S-slices and ltgt-slices (logit at
     target), each matmul writing one partition row of a single
     (128,468) PSUM tile (rows 0-63 = S, 64-127 = ltgt).
  4. One ACT copy evacuates the (128,468) PSUM tile; focal epilogue on
     (64,468): z = ltgt - ln S, p = exp(z), loss = C0*(1-p)^2*z*w with
     w = 13 if t<79.5 else 1 (bg pixels always bin 80). Free-dim
     accumulate -> (64,1) partials summed on host across cores.
"""

import numpy as np
from contextlib import ExitStack

import concourse.bass as bass
import concourse.bacc as bacc_mod
import concourse.tile as tile
import concourse.mybir as mybir
from concourse.bass_utils import run_bass_kernel_spmd

try:
    import ml_dtypes

    BF16 = ml_dtypes.bfloat16
except ImportError:  # pragma: no cover
    BF16 = None

# Problem constants (hardcoded per contract)
B, C, H, W, N = 8, 81, 96, 312, 32
HW = H * W                      # 29952
CHUNK = 3744                    # pixels per streamed chunk
NCHUNK = HW // CHUNK            # 8
SLICE = 468                     # matmul N per PSUM-bank tile
NSL = CHUNK // SLICE            # 8 slices per chunk
EPP = 64                        # epilogue partitions (64 x 468 = HW)

ALPHA = 0.25
D_MIN, D_MAX, NUM_BINS = 0.001, 60.0, 80
BIN_SIZE = 2.0 * (D_MAX - D_MIN) / (NUM_BINS * (1 + NUM_BINS))
K1 = 8.0 / BIN_SIZE             # sqrt arg scale
B1 = 1.0 - K1 * D_MIN           # sqrt arg bias
BIG = 3.0e4                     # additive sentinel; 2*BIG fits fp16
C0 = -ALPHA / float(B * HW)     # fold -alpha and global pixel normalizer

LAST_RESULTS = None


def build_program():
    f32 = mybir.dt.float32
    f16 = mybir.dt.float16
    bf16 = mybir.dt.bfloat16
    i32 = mybir.dt.int32
    Alu = mybir.AluOpType
    Act = mybir.ActivationFunctionType

    nc = bacc_mod.Bacc("TRN2", target_bir_lowering=False)
    logits = nc.dram_tensor("logits", [C, HW], bf16, kind="ExternalInput")
    rowpen = nc.dram_tensor("rowpen", [H, N], f32, kind="ExternalInput")
    colval = nc.dram_tensor("colval", [N, W], f32, kind="ExternalInput")
    iota81 = nc.dram_tensor("iota81", [C, 1], f32, kind="ExternalInput")
    ones96 = nc.dram_tensor("ones96", [1, H], f32, kind="ExternalInput")
    onehot = nc.dram_tensor("onehot", [C, 63], bf16, kind="ExternalInput")
    partial = nc.dram_tensor("partial", [EPP, 1], f32, kind="ExternalOutput")
    tprobe = nc.dram_tensor("tprobe", [1, HW], f32, kind="ExternalOutput")

    with ExitStack() as ctx:
        tc = ctx.enter_context(tile.TileContext(nc))
        consts = ctx.enter_context(tc.tile_pool(name="consts", bufs=1))
        rast = ctx.enter_context(tc.tile_pool(name="rast", bufs=1))
        candp = ctx.enter_context(tc.tile_pool(name="cand", bufs=4))
        ts_pool = ctx.enter_context(tc.tile_pool(name="tstage", bufs=1))
        lg = ctx.enter_context(tc.tile_pool(name="lg", bufs=2))
        xp = ctx.enter_context(tc.tile_pool(name="xp", bufs=2))
        mlp = ctx.enter_context(tc.tile_pool(name="ml", bufs=2))
        tbp = ctx.enter_context(tc.tile_pool(name="tb", bufs=2))
        fin = ctx.enter_context(tc.tile_pool(name="fin", bufs=1))
        psR = ctx.enter_context(tc.tile_pool(name="psR", bufs=4, space="PSUM"))
        psSE = ctx.enter_context(tc.tile_pool(name="psSE", bufs=1, space="PSUM"))
        dr = ctx.enter_context(tc.tile_pool(name="dr", bufs=1, space="DRAM"))

        # ---- constants
        zero128 = consts.tile([128, 1], f32)
        nc.vector.memset(zero128[:], 0.0)
        nc.const_aps.aps[(f32, 0.0)] = zero128[:]
        b1t = consts.tile([128, 1], f32)
        nc.vector.memset(b1t[:], B1)
        nc.const_aps.aps[(f32, B1)] = b1t[:]

        c_iota = consts.tile([C, 1], f32)
        nc.sync.dma_start(c_iota[:], iota81[:, :])
        c_ones96 = consts.tile([1, H], f32)
        nc.sync.dma_start(c_ones96[:], ones96[:, :])
        c_oh = consts.tile([C, 63], bf16)
        nc.sync.dma_start(c_oh[:], onehot[:, :])
        c_rowpen = consts.tile([H, N], f32)
        nc.sync.dma_start(c_rowpen[:], rowpen[:, :])
        c_cv = []
        for n in range(N):
            cvn = consts.tile([1, W], f32, tag=f"cv{n}")
            nc.sync.dma_start(cvn[:], colval[n:n + 1, :])
            c_cv.append(cvn)

        # ---- rasterize: dmin(h,w) = min_n relu(colval(n,w) + rowpen(h,n))
        dmin = rast.tile([H, W], f16)
        nc.vector.memset(dmin[:], BIG)
        for n in range(N):
            bc = psR.tile([H, W], f32, tag="bc")
            nc.tensor.matmul(bc[:], c_ones96[:, :], c_cv[n][0:1, :],
                             start=True, stop=True)
            cand = candp.tile([H, W], f16, tag="cand")
            nc.scalar.activation(cand[:], bc[:], Act.Relu,
                                 bias=c_rowpen[:, n:n + 1])
            nc.vector.tensor_tensor(out=dmin[:], in0=dmin[:], in1=cand[:],
                                    op=Alu.min)

        # ---- per-pixel targets in raster layout (96,312), fp32
        fg = ts_pool.tile([H, W], f32)
        nc.vector.tensor_scalar(out=fg[:], in0=dmin[:], scalar1=BIG * 0.5,
                                scalar2=None, op0=Alu.is_lt)
        deff = ts_pool.tile([H, W], f32)
        nc.vector.tensor_tensor(out=deff[:], in0=dmin[:], in1=fg[:],
                                op=Alu.mult)
        # idx = 0.5*sqrt(K1*d + B1) - 0.5
        sq = ts_pool.tile([H, W], f32)
        nc.scalar.activation(sq[:], deff[:], Act.Sqrt, bias=B1, scale=K1)
        idx = ts_pool.tile([H, W], f32)
        nc.vector.tensor_scalar(out=idx[:], in0=sq[:], scalar1=0.5,
                                scalar2=-0.5, op0=Alu.mult, op1=Alu.add)
        neg = ts_pool.tile([H, W], f32)
        nc.vector.tensor_scalar(out=neg[:], in0=idx[:], scalar1=0.0,
                                scalar2=None, op0=Alu.is_lt)
        # floor for idx>=0 == rne(idx - 0.5) away from exact halves
        idxc = ts_pool.tile([H, W], f32)
        nc.vector.tensor_scalar(out=idxc[:], in0=idx[:], scalar1=80.0,
                                scalar2=-0.5, op0=Alu.min, op1=Alu.add)
        ti = ts_pool.tile([H, W], i32)
        nc.vector.tensor_copy(out=ti[:], in_=idxc[:])
        tf = ts_pool.tile([H, W], f32)
        nc.vector.tensor_copy(out=tf[:], in_=ti[:])
        # t = tf + neg*(80 - tf)   (idx<0 -> bin 80)
        d80 = ts_pool.tile([H, W], f32)
        nc.vector.tensor_scalar(out=d80[:], in0=tf[:], scalar1=-1.0,
                                scalar2=80.0, op0=Alu.mult, op1=Alu.add)
        nd = ts_pool.tile([H, W], f32)
        nc.vector.tensor_tensor(out=nd[:], in0=neg[:], in1=d80[:], op=Alu.mult)
        tt_ = ts_pool.tile([H, W], f32)
        nc.vector.tensor_tensor(out=tt_[:], in0=tf[:], in1=nd[:], op=Alu.add)
        tbf = ts_pool.tile([H, W], bf16)
        nc.vector.tensor_copy(out=tbf[:], in_=tt_[:])

        nc.sync.dma_start(tprobe[0:1, :], tt_[:])

        # ---- bounce t row to DRAM (flat pixel order)
        t_scr = dr.tile([1, HW], bf16)
        nc.sync.dma_start(t_scr[:, :], tbf[:])

        # ---- stream logits; batched exp + select; PE ones-reductions
        SEs_ = psSE.tile([EPP, SLICE], f32, tag="S")   # row r: S slice r
        SEe_ = psSE.tile([EPP, SLICE], f32, tag="E")   # row r: ltgt slice r
        for j in range(NCHUNK):
            sl = slice(j * CHUNK, (j + 1) * CHUNK)
            L = lg.tile([C, CHUNK], bf16, tag="L")
            nc.sync.dma_start(L[:], logits[:, sl])
            tb = tbp.tile([C, CHUNK], bf16, tag="tb")
            nc.sync.dma_start(tb[:], t_scr[0:1, sl].partition_broadcast(C))
            X = xp.tile([C, CHUNK], bf16, tag="X")
            nc.scalar.activation(X[:], L[:], Act.Exp)
            mL = mlp.tile([C, CHUNK], bf16, tag="mL")
            nc.vector.scalar_tensor_tensor(
                out=mL[:], in0=tb[:], scalar=c_iota[:, 0:1], in1=L[:],
                op0=Alu.is_equal, op1=Alu.mult)
            for k in range(NSL):
                r = j * NSL + k
                blk, m = divmod(r, 32)
                first, last = (m == 0), (m == 31)
                lhsT = c_oh[:, 31 - m:63 - m]
                ksl = slice(k * SLICE, (k + 1) * SLICE)
                nc.tensor.matmul(SEs_[32 * blk:32 * blk + 32, :], lhsT,
                                 X[:, ksl], start=first, stop=last,
                                 skip_group_check=True)
                nc.tensor.matmul(SEe_[32 * blk:32 * blk + 32, :], lhsT,
                                 mL[:, ksl], start=first, stop=last,
                                 skip_group_check=True)

        # ---- evacuate PSUM in two shots; epilogue on (64,468)
        Ssb = fin.tile([EPP, SLICE], f32)
        nc.scalar.activation(Ssb[:], SEs_[:], Act.Copy)
        Esb = fin.tile([EPP, SLICE], f32)
        nc.scalar.activation(Esb[:], SEe_[:], Act.Copy)
        S_ = Ssb[:]
        E_ = Esb[:]

        tsl = fin.tile([EPP, SLICE], bf16)
        nc.sync.dma_start(
            tsl[:], t_scr[0:1, :].rearrange("a (p f) -> (a p) f", p=EPP))
        w1 = fin.tile([EPP, SLICE], f32)
        nc.vector.tensor_scalar(out=w1[:], in0=tsl[:], scalar1=79.5,
                                scalar2=None, op0=Alu.is_lt)
        wgt = fin.tile([EPP, SLICE], f32)
        nc.vector.tensor_scalar(out=wgt[:], in0=w1[:], scalar1=12.0,
                                scalar2=1.0, op0=Alu.mult, op1=Alu.add)

        lnS = fin.tile([EPP, SLICE], f32)
        nc.scalar.activation(lnS[:], S_, Act.Ln)
        z = fin.tile([EPP, SLICE], f32)
        nc.vector.tensor_tensor(out=z[:], in0=E_, in1=lnS[:], op=Alu.subtract)
        p = fin.tile([EPP, SLICE], f32)
        nc.scalar.activation(p[:], z[:], Act.Exp)
        om = fin.tile([EPP, SLICE], f32)
        nc.vector.tensor_scalar(out=om[:], in0=p[:], scalar1=-1.0,
                                scalar2=1.0, op0=Alu.mult, op1=Alu.add)
        om2 = fin.tile([EPP, SLICE], f32)
        nc.vector.tensor_tensor(out=om2[:], in0=om[:], in1=om[:], op=Alu.mult)
        t2 = fin.tile([EPP, SLICE], f32)
        nc.vector.scalar_tensor_tensor(
            out=t2[:], in0=om2[:], scalar=C0, in1=z[:],
            op0=Alu.mult, op1=Alu.mult)
        fs = fin.tile([EPP, SLICE], f32)
        acc = fin.tile([EPP, 1], f32)
        nc.vector.scalar_tensor_tensor(
            out=fs[:], in0=t2[:], scalar=0.0, in1=wgt[:],
            op0=Alu.add, op1=Alu.mult, accum_out=acc[:])
        nc.sync.dma_start(partial[:, :], acc[:])

    nc.compile()
    return nc


_CACHE = {}


def _get_program():
    if "nc" not in _CACHE:
        _CACHE["nc"] = build_program()
    return _CACHE["nc"]


def kernel(depth_logits, gt_boxes2d, num_gt_per_img, gt_center_depth):
    global LAST_RESULTS
    dl = np.asarray(depth_logits, dtype=np.float32)
    assert dl.shape == (B, C, H, W), dl.shape
    n_gt = int(num_gt_per_img)
    assert n_gt == N, n_gt
    boxes = np.asarray(gt_boxes2d, dtype=np.float32)
    depth = np.asarray(gt_center_depth, dtype=np.float32)

    u1 = np.floor(boxes[:, 0]).astype(np.int32)
    v1 = np.floor(boxes[:, 1]).astype(np.int32)
    u2 = np.ceil(boxes[:, 2]).astype(np.int32)
    v2 = np.ceil(boxes[:, 3]).astype(np.int32)
    rows = np.arange(H)[:, None]
    cols = np.arange(W)[None, :]
    iota = np.arange(C, dtype=np.float32).reshape(C, 1)
    ones96_a = np.ones((1, H), dtype=np.float32)
    onehot_a = np.zeros((C, 63), dtype=BF16)
    onehot_a[:, 31] = 1.0

    logits_bf = np.ascontiguousarray(dl.reshape(B, C, HW).astype(BF16))
    in_maps = []
    for b in range(B):
        sl = slice(b * N, (b + 1) * N)
        bv1, bv2, bu1, bu2, d = v1[sl], v2[sl], u1[sl], u2[sl], depth[sl]
        rp = np.where((rows >= bv1[None, :]) & (rows < bv2[None, :]),
                      0.0, BIG).astype(np.float32)              # (H, N)
        cv = np.where((cols >= bu1[:, None]) & (cols < bu2[:, None]),
                      d[:, None], BIG).astype(np.float32)       # (N, W)
        in_maps.append({
            "logits": logits_bf[b],
            "rowpen": np.ascontiguousarray(rp),
            "colval": np.ascontiguousarray(cv),
            "iota81": iota,
            "ones96": ones96_a,
            "onehot": onehot_a,
        })

    nc = _get_program()
    res = run_bass_kernel_spmd(nc, in_maps, core_ids=list(range(B)))
    LAST_RESULTS = res
    total = np.float64(0.0)
    for r in res.results:
        total += np.asarray(r["partial"], dtype=np.float64).sum()
    return np.float32(total)


if __name__ == "__main__":
    import tempfile
    from concourse.bass_utils import compile_bass_kernel
    compile_bass_kernel(_get_program(), tempfile.mkdtemp())
    print("COMPILE OK")


# revision 13
# speedup vs baseline: 1.9131x; 1.4539x over previous
"""DDNLoss (depth distribution network focal loss) on 8 trn2 NeuronCores.

v3 — natural-layout, batched, PE-light raster. Data-parallel over B
(1 image per core).

Per core:
  1. Raster: host ships 8 quad-min candidate maps (96, 8*312) bf16
     (additive BIG sentinels); device tree-mins them in 3 wide DVE ops
     -> dmin (96,312) bf16.
  2. LID bin targets t(h,w) as the torch reference (floor via RNE cast
     of idx-0.5); t bounced to DRAM as a bf16 flat row.
  3. Logits streamed as bf16 (host-converted) in 4 chunks of (81,7488).
     Per chunk: ACT exp -> X; DVE tensor_scalar is_equal against the
     per-partition channel index builds the one-hot mask from the
     0-stride broadcast DMA of the t row; DVE tensor_tensor mult ->
     masked_L. PE reduces X and masked_L with one-hot-column (81,32)
     stationaries accumulating into two (64,468) PSUM tiles (row r =
     pixel slice r) -> S and ltgt per pixel.
  4. Two ACT copies evacuate PSUM; focal epilogue on (64,468):
     z = ltgt - ln S, p = exp(z), loss = C0*(1-p)^2*z*w with w = 13 if
     t<79.5 else 1 (bg pixels are always bin 80). Free-dim accumulate
     -> (64,1) partials summed on host across cores.
"""

import numpy as np
from contextlib import ExitStack

import concourse.bass as bass
import concourse.bacc as bacc_mod
import concourse.tile as tile
import concourse.mybir as mybir
from concourse.bass_utils import run_bass_kernel_spmd

try:
    import ml_dtypes

    BF16 = ml_dtypes.bfloat16
except ImportError:  # pragma: no cover
    BF16 = None

# Problem constants (hardcoded per contract)
B, C, H, W, N = 8, 81, 96, 312, 32
HW = H * W                      # 29952
CHUNK = 7488                    # pixels per streamed chunk
NCHUNK = HW // CHUNK            # 4
SLICE = 468                     # matmul N per PSUM-bank tile
NSL = CHUNK // SLICE            # 16 slices per chunk
EPP = 64                        # epilogue partitions (64 x 468 = HW)
NQUAD = 8                       # host-premined candidate quads

ALPHA = 0.25
D_MIN, D_MAX, NUM_BINS = 0.001, 60.0, 80
BIN_SIZE = 2.0 * (D_MAX - D_MIN) / (NUM_BINS * (1 + NUM_BINS))
K1 = 8.0 / BIN_SIZE             # sqrt arg scale
B1 = 1.0 - K1 * D_MIN           # sqrt arg bias
BIG = 3.0e4                     # additive sentinel
C0 = -ALPHA / float(B * HW)     # fold -alpha and global pixel normalizer

LAST_RESULTS = None


def build_program():
    f32 = mybir.dt.float32
    bf16 = mybir.dt.bfloat16
    i32 = mybir.dt.int32
    Alu = mybir.AluOpType
    Act = mybir.ActivationFunctionType

    nc = bacc_mod.Bacc("TRN2", target_bir_lowering=False)
    logits = nc.dram_tensor("logits", [C, HW], bf16, kind="ExternalInput")
    cands = nc.dram_tensor("cands", [H, NQUAD * W], bf16, kind="ExternalInput")
    iota81 = nc.dram_tensor("iota81", [C, 1], f32, kind="ExternalInput")
    onehot = nc.dram_tensor("onehot", [C, 63], bf16, kind="ExternalInput")
    partial = nc.dram_tensor("partial", [EPP, 1], f32, kind="ExternalOutput")
    tprobe = nc.dram_tensor("tprobe", [1, HW], f32, kind="ExternalOutput")

    with ExitStack() as ctx:
        tc = ctx.enter_context(tile.TileContext(nc))
        consts = ctx.enter_context(tc.tile_pool(name="consts", bufs=1))
        rast = ctx.enter_context(tc.tile_pool(name="rast", bufs=1))
        ts_pool = ctx.enter_context(tc.tile_pool(name="tstage", bufs=1))
        lg = ctx.enter_context(tc.tile_pool(name="lg", bufs=2))
        xp = ctx.enter_context(tc.tile_pool(name="xp", bufs=2))
        mkp = ctx.enter_context(tc.tile_pool(name="mk", bufs=2))
        mlp = ctx.enter_context(tc.tile_pool(name="ml", bufs=2))
        tbp = ctx.enter_context(tc.tile_pool(name="tb", bufs=2))
        fin = ctx.enter_context(tc.tile_pool(name="fin", bufs=1))
        psSE = ctx.enter_context(tc.tile_pool(name="psSE", bufs=1, space="PSUM"))
        dr = ctx.enter_context(tc.tile_pool(name="dr", bufs=1, space="DRAM"))

        # ---- const APs for activation float biases
        zero128 = consts.tile([128, 1], f32)
        nc.vector.memset(zero128[:], 0.0)
        nc.const_aps.aps[(f32, 0.0)] = zero128[:]
        b1t = consts.tile([128, 1], f32)
        nc.vector.memset(b1t[:], B1)
        nc.const_aps.aps[(f32, B1)] = b1t[:]

        c_iota = consts.tile([C, 1], f32)
        nc.sync.dma_start(c_iota[:], iota81[:, :])
        c_oh = consts.tile([C, 63], bf16)
        nc.sync.dma_start(c_oh[:], onehot[:, :])

        # ---- raster: tree-min the 8 host-premined quad candidate maps
        cq = rast.tile([H, NQUAD * W], bf16)
        nc.sync.dma_start(cq[:], cands[:, :])
        h1 = rast.tile([H, 4 * W], bf16)
        nc.vector.tensor_tensor(out=h1[:], in0=cq[:, 0:4 * W],
                                in1=cq[:, 4 * W:8 * W], op=Alu.min)
        h2 = rast.tile([H, 2 * W], bf16)
        nc.vector.tensor_tensor(out=h2[:], in0=h1[:, 0:2 * W],
                                in1=h1[:, 2 * W:4 * W], op=Alu.min)
        dmin = rast.tile([H, W], bf16)
        nc.vector.tensor_tensor(out=dmin[:], in0=h2[:, 0:W],
                                in1=h2[:, W:2 * W], op=Alu.min)

        # ---- per-pixel targets in raster layout (96,312), fp32
        fg = ts_pool.tile([H, W], f32)
        nc.vector.tensor_scalar(out=fg[:], in0=dmin[:], scalar1=BIG * 0.5,
                                scalar2=None, op0=Alu.is_lt)
        deff = ts_pool.tile([H, W], f32)
        nc.vector.tensor_tensor(out=deff[:], in0=dmin[:], in1=fg[:],
                                op=Alu.mult)
        # idx = 0.5*sqrt(K1*d + B1) - 0.5 ; neg <=> sqrt arg < 1
        sq = ts_pool.tile([H, W], f32)
        nc.scalar.activation(sq[:], deff[:], Act.Sqrt, bias=B1, scale=K1)
        neg = ts_pool.tile([H, W], f32)
        nc.vector.tensor_scalar(out=neg[:], in0=sq[:], scalar1=1.0,
                                scalar2=None, op0=Alu.is_lt)
        # floor(min(idx,80)) for idx>=0 == rne(min(0.5*sq-0.5, 80) - 0.5)
        idxc = ts_pool.tile([H, W], f32)
        nc.vector.tensor_scalar(out=idxc[:], in0=sq[:], scalar1=0.5,
                                scalar2=-1.0, op0=Alu.mult, op1=Alu.add)
        idxm = ts_pool.tile([H, W], f32)
        nc.vector.tensor_scalar(out=idxm[:], in0=idxc[:], scalar1=79.5,
                                scalar2=None, op0=Alu.min)
        ti = ts_pool.tile([H, W], i32)
        nc.vector.tensor_copy(out=ti[:], in_=idxm[:])
        tf = ts_pool.tile([H, W], f32)
        nc.vector.tensor_copy(out=tf[:], in_=ti[:])
        # t = max(tf, neg*80)   (idx<0 -> bin 80; else tf in [0,80])
        tt_ = ts_pool.tile([H, W], f32)
        nc.vector.scalar_tensor_tensor(
            out=tt_[:], in0=neg[:], scalar=80.0, in1=tf[:],
            op0=Alu.mult, op1=Alu.max)
        tbf = ts_pool.tile([H, W], bf16)
        nc.vector.tensor_copy(out=tbf[:], in_=tt_[:])

        nc.sync.dma_start(tprobe[0:1, :], tt_[:])

        # ---- bounce t row to DRAM (flat pixel order)
        t_scr = dr.tile([1, HW], bf16)
        nc.sync.dma_start(t_scr[:, :], tbf[:])

        # ---- stream logits; batched exp + select; PE ones-reductions
        SEs_ = psSE.tile([EPP, SLICE], f32, tag="S")   # row r: S slice r
        SEe_ = psSE.tile([EPP, SLICE], f32, tag="E")   # row r: ltgt slice r
        for j in range(NCHUNK):
            sl = slice(j * CHUNK, (j + 1) * CHUNK)
            L = lg.tile([C, CHUNK], bf16, tag="L")
            nc.sync.dma_start(L[:], logits[:, sl])
            tb = tbp.tile([C, CHUNK], bf16, tag="tb")
            nc.gpsimd.dma_start(tb[:], t_scr[0:1, sl].partition_broadcast(C))
            X = xp.tile([C, CHUNK], bf16, tag="X")
            nc.scalar.activation(X[:], L[:], Act.Exp)
            mk = mkp.tile([C, CHUNK], bf16, tag="mk")
            nc.vector.tensor_scalar(out=mk[:], in0=tb[:],
                                    scalar1=c_iota[:, 0:1], scalar2=None,
                                    op0=Alu.is_equal)
            mL = mlp.tile([C, CHUNK], bf16, tag="mL")
            nc.vector.tensor_tensor(out=mL[:], in0=mk[:], in1=L[:],
                                    op=Alu.mult)
            for k in range(NSL):
                r = j * NSL + k
                blk, m = divmod(r, 32)
                first, last = (m == 0), (m == 31)
                lhsT = c_oh[:, 31 - m:63 - m]
                ksl = slice(k * SLICE, (k + 1) * SLICE)
                nc.tensor.matmul(SEs_[32 * blk:32 * blk + 32, :], lhsT,
                                 X[:, ksl], start=first, stop=last,
                                 skip_group_check=True)
                nc.tensor.matmul(SEe_[32 * blk:32 * blk + 32, :], lhsT,
                                 mL[:, ksl], start=first, stop=last,
                                 skip_group_check=True)

        # ---- evacuate PSUM; epilogue on (64,468)
        Ssb = fin.tile([EPP, SLICE], f32)
        nc.scalar.activation(Ssb[:], SEs_[:], Act.Copy)
        Esb = fin.tile([EPP, SLICE], f32)
        nc.scalar.activation(Esb[:], SEe_[:], Act.Copy)

        tsl = fin.tile([EPP, SLICE], bf16)
        nc.sync.dma_start(
            tsl[:], t_scr[0:1, :].rearrange("a (p f) -> (a p) f", p=EPP))
        w1 = fin.tile([EPP, SLICE], f32)
        nc.vector.tensor_scalar(out=w1[:], in0=tsl[:], scalar1=79.5,
                                scalar2=None, op0=Alu.is_lt)
        wgt = fin.tile([EPP, SLICE], f32)
        nc.vector.tensor_scalar(out=wgt[:], in0=w1[:], scalar1=12.0,
                                scalar2=1.0, op0=Alu.mult, op1=Alu.add)

        lnS = fin.tile([EPP, SLICE], f32)
        nc.scalar.activation(lnS[:], Ssb[:], Act.Ln)
        z = fin.tile([EPP, SLICE], f32)
        nc.vector.tensor_tensor(out=z[:], in0=Esb[:], in1=lnS[:],
                                op=Alu.subtract)
        p = fin.tile([EPP, SLICE], f32)
        nc.scalar.activation(p[:], z[:], Act.Exp)
        om = fin.tile([EPP, SLICE], f32)
        nc.vector.tensor_scalar(out=om[:], in0=p[:], scalar1=-1.0,
                                scalar2=1.0, op0=Alu.mult, op1=Alu.add)
        om2 = fin.tile([EPP, SLICE], f32)
        nc.vector.tensor_tensor(out=om2[:], in0=om[:], in1=om[:], op=Alu.mult)
        t2 = fin.tile([EPP, SLICE], f32)
        nc.vector.scalar_tensor_tensor(
            out=t2[:], in0=om2[:], scalar=C0, in1=z[:],
            op0=Alu.mult, op1=Alu.mult)
        fs = fin.tile([EPP, SLICE], f32)
        acc = fin.tile([EPP, 1], f32)
        nc.vector.scalar_tensor_tensor(
            out=fs[:], in0=t2[:], scalar=0.0, in1=wgt[:],
            op0=Alu.add, op1=Alu.mult, accum_out=acc[:])
        nc.sync.dma_start(partial[:, :], acc[:])

    nc.compile()
    return nc


_CACHE = {}


def _get_program():
    if "nc" not in _CACHE:
        _CACHE["nc"] = build_program()
    return _CACHE["nc"]


def kernel(depth_logits, gt_boxes2d, num_gt_per_img, gt_center_depth):
    global LAST_RESULTS
    dl = np.asarray(depth_logits, dtype=np.float32)
    assert dl.shape == (B, C, H, W), dl.shape
    n_gt = int(num_gt_per_img)
    assert n_gt == N, n_gt
    boxes = np.asarray(gt_boxes2d, dtype=np.float32)
    depth = np.asarray(gt_center_depth, dtype=np.float32)

    u1 = np.floor(boxes[:, 0]).astype(np.int32)
    v1 = np.floor(boxes[:, 1]).astype(np.int32)
    u2 = np.ceil(boxes[:, 2]).astype(np.int32)
    v2 = np.ceil(boxes[:, 3]).astype(np.int32)
    rows = np.arange(H)[:, None]
    cols = np.arange(W)[None, :]
    iota = np.arange(C, dtype=np.float32).reshape(C, 1)
    onehot_a = np.zeros((C, 63), dtype=BF16)
    onehot_a[:, 31] = 1.0

    logits_bf = np.ascontiguousarray(dl.reshape(B, C, HW).astype(BF16))
    in_maps = []
    for b in range(B):
        sl = slice(b * N, (b + 1) * N)
        bv1, bv2, bu1, bu2, d = v1[sl], v2[sl], u1[sl], u2[sl], depth[sl]
        rp = np.where((rows >= bv1[None, :]) & (rows < bv2[None, :]),
                      0.0, BIG).astype(np.float32)              # (H, N)
        cv = np.where((cols >= bu1[:, None]) & (cols < bu2[:, None]),
                      d[:, None], BIG).astype(np.float32)       # (N, W)
        # cand(n,h,w) = rp(h,n) + cv(n,w); quad-min over groups of 4 boxes
        cand = rp.T[:, :, None] + cv[:, None, :]                # (N, H, W)
        quad = cand.reshape(NQUAD, 4, H, W).min(axis=1)         # (8, H, W)
        cands_a = np.ascontiguousarray(
            quad.transpose(1, 0, 2).reshape(H, NQUAD * W).astype(BF16))
        in_maps.append({
            "logits": logits_bf[b],
            "cands": cands_a,
            "iota81": iota,
            "onehot": onehot_a,
        })

    nc = _get_program()
    res = run_bass_kernel_spmd(nc, in_maps, core_ids=list(range(B)))
    LAST_RESULTS = res
    total = np.float64(0.0)
    for r in res.results:
        total += np.asarray(r["partial"], dtype=np.float64).sum()
    return np.float32(total)


if __name__ == "__main__":
    import tempfile
    from concourse.bass_utils import compile_bass_kernel
    compile_bass_kernel(_get_program(), tempfile.mkdtemp())
    print("COMPILE OK")
